# revision 5
# baseline (speedup 1.0000x reference)
"""Trainium2 Bass kernel for the GRU decoder (teacher-forced) + output
projection + log_softmax.

Problem shapes: B=32, T=82, H=128, V=32000.
  log_probs [B, T, V] f32  (~336 MB -> memory-bound on the output write)
  hidden    [1, B, H] f32

Sharding: data-parallel over batch across 8 cores (4 sequences/core).
Each core:
  - gathers+relus its token embeddings (indirect DMA from emb table)
  - x_proj = x @ W_ih.T (+biases folded) as [H,3,R] tiles (bf16 matmuls)
  - runs the 82-step GRU scan in [H, B_loc] orientation; sigmoid computed
    via tanh (sigma(x) = 0.5 + 0.5*tanh(x/2)) so the ACT engine never
    switches activation tables (exp/tanh live in one table set)
  - output projection in [rows, V] orientation, 128-row tiles, 1536-wide
    vocab groups: pass 1 computes sum(exp(logits+b)) via ACT accum_out;
    log(sum) is computed with an exponent-extraction init + 2 Newton
    iterations (avoids the Ln table set); pass 2 recomputes logits and a
    single fused DVE op emits (logits - logsumexp) + b into the staging
    tile which DMAs straight out.
"""

import numpy as np
import ml_dtypes

import concourse.bacc as bacc
import concourse.bass as bass
import concourse.mybir as mybir
import concourse.tile as tile
from concourse.bass_utils import run_bass_kernel_spmd
from concourse.masks import make_identity

F32 = mybir.dt.float32
BF16 = mybir.dt.bfloat16
I32 = mybir.dt.int32
AF = mybir.ActivationFunctionType
OP = mybir.AluOpType
AX = mybir.AxisListType

B, T, H, V = 32, 82, 128, 32000
NCORES = 8
BL = B // NCORES            # 4 sequences per core
R = BL * T                  # 328 (t-major rows: r = 4*t + b)
MKS = (128, 128, 72)        # row-tile sizes
GW = 1536                   # vocab group width (3 PSUM banks)
GROUPS = [(v0, min(GW, V - v0)) for v0 in range(0, V, GW)]
LN2 = 0.6931471805599453

_NC = None


def _emit(nc, tc, ctx, d):
    consts = ctx.enter_context(tc.tile_pool(name="consts", bufs=1))
    work = ctx.enter_context(tc.tile_pool(name="work", bufs=2))
    stage_p = ctx.enter_context(tc.tile_pool(name="stagep", bufs=3))
    ps_big = ctx.enter_context(tc.tile_pool(name="psbig", bufs=2, space="PSUM"))
    ps_sm = ctx.enter_context(tc.tile_pool(name="pssm", bufs=2, space="PSUM"))

    # ---- constants into SBUF ----
    whh = consts.tile([H, 3 * H], BF16, tag="whh")
    nc.sync.dma_start(out=whh[:, :], in_=d["whht"][:, :])
    wih = consts.tile([H, 3 * H], BF16, tag="wih")
    nc.sync.dma_start(out=wih[:, :], in_=d["wiht"][:, :])
    bsc = consts.tile([H, 4], F32, tag="bsc")
    nc.sync.dma_start(out=bsc[:, :], in_=d["bsc"][:, :])
    wo = consts.tile([H, V], BF16, tag="wo")
    for i in range(8):
        s = slice(i * (V // 8), (i + 1) * (V // 8))
        nc.sync.dma_start(out=wo[:, s], in_=d["wot"][:, s])
    # bias broadcast [H, V]: load one row then doubling copies (on-chip)
    bb = consts.tile([H, V], BF16, tag="bb")
    nc.sync.dma_start(out=bb[0:1, :], in_=d["bo"][0:1, :])
    p = 1
    while p < H:
        nc.sync.dma_start(out=bb[p:2 * p, :], in_=bb[0:p, :])
        p *= 2

    h0f = consts.tile([H, BL], F32, tag="h0f")
    nc.sync.dma_start(out=h0f[:, :], in_=d["h0t"][:, :])
    h0b = consts.tile([H, BL], BF16, tag="h0b")
    nc.vector.tensor_copy(out=h0b[:, :], in_=h0f[:, :])

    ones_bf = consts.tile([1, H], BF16, tag="ones")
    nc.vector.memset(ones_bf[:, :], 1.0)
    idb = consts.tile([H, H], BF16, tag="idb")
    make_identity(nc, idb[:, :])
    idf = consts.tile([H, H], F32, tag="idf")
    make_identity(nc, idf[:, :])

    # ---- embedding gather + relu + transpose ----
    xT = consts.tile([H, R], BF16, tag="xT")   # x^T, relu'd, bf16
    for k in range(3):
        mk = MKS[k]
        base = 128 * k
        tokt = consts.tile([128, 1], I32, tag=f"tok{k}")
        nc.sync.dma_start(out=tokt[:mk, 0:1], in_=d["tok"][base:base + mk])
        xf = work.tile([128, H], F32, tag="xf")
        nc.gpsimd.indirect_dma_start(
            out=xf[:mk, :],
            out_offset=None,
            in_=d["emb"][:, :],
            in_offset=bass.IndirectOffsetOnAxis(ap=tokt[:mk, 0:1], axis=0),
        )
        xb = work.tile([128, H], BF16, tag="xb")
        nc.vector.tensor_scalar_max(out=xb[:mk, :], in0=xf[:mk, :], scalar1=0.0)
        pst = ps_sm.tile([128, 384], BF16, tag="ps_sm")
        nc.tensor.transpose(out=pst[:H, :mk], in_=xb[:mk, :H], identity=idb[:mk, :mk])
        nc.vector.tensor_copy(out=xT[:, base:base + mk], in_=pst[:H, :mk])

    # ---- x_proj: xpT[:, g, :] = W_ih_g @ x^T + bias_g ----
    # bsc columns: 0: b_ih_r + b_hh_r, 1: b_ih_z + b_hh_z, 2: b_ih_n,
    #              3: 0.5 * b_hh_n
    xpT = consts.tile([H, 3, R], F32, tag="xpT")
    for g in range(3):
        psx = ps_sm.tile([128, 384], F32, tag="ps_sm")
        nc.tensor.matmul(psx[:H, :R], lhsT=wih[:, g * H:(g + 1) * H],
                         rhs=xT[:, :R], start=True, stop=True)
        nc.scalar.activation(out=xpT[:, g, :], in_=psx[:H, :R],
                             func=AF.Identity, bias=bsc[:, g:g + 1], scale=1.0)

    # ---- GRU scan, 82 steps; h stored [H, BL] ----
    # hbT[k] col 4*j+b = (h after step t=32k+j) for batch lane b, bf16
    hbT = [consts.tile([H, 128], BF16, tag=f"hbT{k}", name=f"hbT{k}")
           for k in range(3)]
    hf_prev = h0f
    for t in range(T):
        if t == 0:
            prev_b = h0b[:, 0:BL]
        else:
            pk, pj = (t - 1) // 32, (t - 1) % 32
            prev_b = hbT[pk][:, 4 * pj:4 * pj + 4]
        ps = ps_sm.tile([128, 384], F32, tag="ps_sm")
        nc.tensor.matmul(ps[:H, 0:4], lhsT=whh[:, 0:H], rhs=prev_b,
                         start=True, stop=True)
        nc.tensor.matmul(ps[:H, 4:8], lhsT=whh[:, H:2 * H], rhs=prev_b,
                         start=True, stop=True)
        nc.tensor.matmul(ps[:H, 8:12], lhsT=whh[:, 2 * H:3 * H], rhs=prev_b,
                         start=True, stop=True)
        # pre_rz = hp_rz + xp_rz   (biases already folded into xpT)
        pre = work.tile([H, 8], F32, tag="pre")
        nc.vector.tensor_add(
            out=pre[:, :].rearrange("p (g b) -> p g b", g=2),
            in0=ps[:H, 0:8].rearrange("p (g b) -> p g b", g=2),
            in1=xpT[:, 0:2, 4 * t:4 * t + 4])
        # tau = tanh(pre/2); sigma(pre) = 0.5 + 0.5*tau
        tau = work.tile([H, 8], F32, tag="tau")
        nc.scalar.activation(out=tau[:, :], in_=pre[:, :], func=AF.Tanh,
                             scale=0.5)
        # hpn_half = 0.5*(hp_n + b_hh_n)
        hpnh = work.tile([H, 4], F32, tag="hpnh")
        nc.scalar.activation(out=hpnh[:, :], in_=ps[:H, 8:12],
                             func=AF.Identity, bias=bsc[:, 3:4], scale=0.5)
        # r*(hp_n + b_hh_n) = (tau_r + 1) * hpn_half
        rhpn = work.tile([H, 4], F32, tag="rhpn")
        nc.vector.scalar_tensor_tensor(out=rhpn[:, :], in0=tau[:, 0:4],
                                       scalar=1.0, in1=hpnh[:, :],
                                       op0=OP.add, op1=OP.mult)
        npre = work.tile([H, 4], F32, tag="npre")
        nc.vector.tensor_add(out=npre[:, :], in0=rhpn[:, :],
                             in1=xpT[:, 2, 4 * t:4 * t + 4])
        nn = work.tile([H, 4], F32, tag="nn")
        nc.scalar.activation(out=nn[:, :], in_=npre[:, :], func=AF.Tanh)
        # h' = n + z*(h - n),  z*(h-n) = 0.5*(tau_z + 1)*(h - n)
        dd = work.tile([H, 4], F32, tag="dd")
        nc.vector.tensor_sub(out=dd[:, :], in0=hf_prev[:, 0:4], in1=nn[:, :])
        ee = work.tile([H, 4], F32, tag="ee")
        nc.vector.scalar_tensor_tensor(out=ee[:, :], in0=tau[:, 4:8],
                                       scalar=1.0, in1=dd[:, :],
                                       op0=OP.add, op1=OP.mult)
        hf = work.tile([H, 4], F32, tag="hf")
        nc.vector.scalar_tensor_tensor(out=hf[:, :], in0=ee[:, :], scalar=0.5,
                                       in1=nn[:, :], op0=OP.mult, op1=OP.add)
        k, j = t // 32, t % 32
        nc.vector.tensor_copy(out=hbT[k][:, 4 * j:4 * j + 4], in_=hf[:, :])
        hf_prev = hf

    # ---- output projection + log_softmax, per 128-row tile ----
    for k in range(3):
        mk = MKS[k]
        base = 128 * k
        lhsT = hbT[k][:, 0:mk]
        acc = consts.tile([128, len(GROUPS)], F32, tag=f"acc{k}")
        # pass 1: sumexp(logits + b) per row
        for gi, (v0, ng) in enumerate(GROUPS):
            ps = ps_big.tile([128, GW], F32, tag="proj")
            for o in range(0, ng, 512):
                n = min(512, ng - o)
                nc.tensor.matmul(ps[:mk, o:o + n], lhsT=lhsT,
                                 rhs=wo[:, v0 + o:v0 + o + n],
                                 start=True, stop=False)
                nc.tensor.matmul(ps[:mk, o:o + n], lhsT=ones_bf[0:1, 0:mk],
                                 rhs=bb[0:1, v0 + o:v0 + o + n],
                                 start=False, stop=True)
            scr = work.tile([128, GW], BF16, tag="scr")
            nc.scalar.activation(out=scr[:mk, 0:ng], in_=ps[:mk, 0:ng],
                                 func=AF.Exp, accum_out=acc[:mk, gi:gi + 1])
        # logsumexp: y = log(S) via exponent-extraction init + 2 Newton steps
        S = consts.tile([128, 1], F32, tag=f"S{k}")
        nc.vector.reduce_sum(out=S[:mk, 0:1], in_=acc[:mk, 0:len(GROUPS)],
                             axis=AX.X)
        y = consts.tile([128, 1], F32, tag=f"y{k}")
        nc.vector.tensor_copy(out=y[:mk, 0:1], in_=S[:mk, 0:1].bitcast(I32))
        nc.vector.tensor_scalar(out=y[:mk, 0:1], in0=y[:mk, 0:1],
                                scalar1=LN2 / (1 << 23),
                                scalar2=-127.0 * LN2,
                                op0=OP.mult, op1=OP.add)
        em = consts.tile([128, 1], F32, tag=f"em{k}")
        for _ in range(2):
            nc.scalar.activation(out=em[:mk, 0:1], in_=y[:mk, 0:1],
                                 func=AF.Exp, scale=-1.0)
            nc.vector.scalar_tensor_tensor(out=y[:mk, 0:1], in0=em[:mk, 0:1],
                                           scalar=S[:mk, 0:1], in1=y[:mk, 0:1],
                                           op0=OP.mult, op1=OP.add)
            nc.vector.tensor_scalar_add(out=y[:mk, 0:1], in0=y[:mk, 0:1],
                                        scalar1=-1.0)
        # pass 2: out = (logits - logsumexp) + b
        for (v0, ng) in GROUPS:
            ps2 = ps_big.tile([128, GW], F32, tag="proj")
            for o in range(0, ng, 512):
                n = min(512, ng - o)
                nc.tensor.matmul(ps2[:mk, o:o + n], lhsT=lhsT,
                                 rhs=wo[:, v0 + o:v0 + o + n],
                                 start=True, stop=True)
            st = stage_p.tile([128, GW], F32, tag="stage")
            nc.vector.scalar_tensor_tensor(out=st[:mk, 0:ng],
                                           in0=ps2[:mk, 0:ng],
                                           scalar=y[:mk, 0:1],
                                           in1=bb[:mk, v0:v0 + ng],
                                           op0=OP.subtract, op1=OP.add)
            nc.sync.dma_start(out=d["olp"][base:base + mk, v0:v0 + ng],
                              in_=st[:mk, 0:ng])

    # ---- final hidden state, transposed to [BL, H] ----
    pso = ps_sm.tile([128, 384], F32, tag="ps_sm")
    nc.tensor.transpose(out=pso[0:BL, 0:H], in_=hf_prev[:, 0:BL],
                        identity=idf[:, :])
    oh_sb = consts.tile([BL, H], F32, tag="oh")
    nc.vector.tensor_copy(out=oh_sb[:, :], in_=pso[0:BL, 0:H])
    nc.sync.dma_start(out=d["oh"][:, :], in_=oh_sb[:, :])


def _build():
    from contextlib import ExitStack
    nc = bacc.Bacc(None, target_bir_lowering=False)
    d = {
        "tok": nc.dram_tensor("tok", [R], I32, kind="ExternalInput"),
        "emb": nc.dram_tensor("emb", [V, H], F32, kind="ExternalInput"),
        "h0t": nc.dram_tensor("h0t", [H, BL], F32, kind="ExternalInput"),
        "wiht": nc.dram_tensor("wiht", [H, 3 * H], BF16, kind="ExternalInput"),
        "whht": nc.dram_tensor("whht", [H, 3 * H], BF16, kind="ExternalInput"),
        "bsc": nc.dram_tensor("bsc", [H, 4], F32, kind="ExternalInput"),
        "wot": nc.dram_tensor("wot", [H, V], BF16, kind="ExternalInput"),
        "bo": nc.dram_tensor("bo", [1, V], BF16, kind="ExternalInput"),
        "olp": nc.dram_tensor("olp", [R, V], F32, kind="ExternalOutput"),
        "oh": nc.dram_tensor("oh", [BL, H], F32, kind="ExternalOutput"),
    }
    with tile.TileContext(nc) as tc:
        with ExitStack() as ctx:
            _emit(nc, tc, ctx, d)
    nc.finalize()
    return nc


def _prepare_in_maps(encoder_hidden, target_tensor, emb,
                     W_ih, W_hh, b_ih, b_hh, W_out, b_out):
    bf16 = ml_dtypes.bfloat16
    tgt = np.asarray(target_tensor)
    tokens = np.concatenate(
        [np.zeros((B, 1), tgt.dtype), tgt[:, :-1]], axis=1).astype(np.int32)
    ehid = np.asarray(encoder_hidden, np.float32)
    embf = np.ascontiguousarray(np.asarray(emb, np.float32))
    b_ih = np.asarray(b_ih, np.float32)
    b_hh = np.asarray(b_hh, np.float32)
    wihT = np.ascontiguousarray(np.asarray(W_ih, np.float32).T).astype(bf16)
    whhT = np.ascontiguousarray(np.asarray(W_hh, np.float32).T).astype(bf16)
    woT = np.ascontiguousarray(np.asarray(W_out, np.float32).T).astype(bf16)
    bo = np.asarray(b_out, np.float32)[None, :].astype(bf16)
    bsc = np.ascontiguousarray(np.stack(
        [b_ih[0:H] + b_hh[0:H],
         b_ih[H:2 * H] + b_hh[H:2 * H],
         b_ih[2 * H:3 * H],
         0.5 * b_hh[2 * H:3 * H]], axis=1).astype(np.float32))

    in_maps = []
    for c in range(NCORES):
        bs = slice(BL * c, BL * (c + 1))
        tok_c = np.ascontiguousarray(tokens[bs].T).reshape(-1)  # t-major [R]
        h0T_c = np.ascontiguousarray(ehid[0, bs, :].T)          # [H, BL]
        in_maps.append({
            "tok": tok_c, "emb": embf, "h0t": h0T_c, "wiht": wihT,
            "whht": whhT, "bsc": bsc, "wot": woT, "bo": bo,
        })
    return in_maps


def _assemble(results):
    lp = np.empty((B, T, V), np.float32)
    hT = np.empty((1, B, H), np.float32)
    for c in range(NCORES):
        r = results[c]
        lp[BL * c:BL * (c + 1)] = \
            r["olp"].reshape(T, BL, V).transpose(1, 0, 2)
        hT[0, BL * c:BL * (c + 1)] = r["oh"]
    return lp, hT


def kernel(encoder_outputs, encoder_hidden, target_tensor, emb,
           W_ih, W_hh, b_ih, b_hh, W_out, b_out):
    global _NC
    if _NC is None:
        _NC = _build()
    in_maps = _prepare_in_maps(encoder_hidden, target_tensor, emb,
                               W_ih, W_hh, b_ih, b_hh, W_out, b_out)
    res = run_bass_kernel_spmd(_NC, in_maps, list(range(NCORES)))
    return _assemble(res.results)


# revision 10
# speedup vs baseline: 1.0075x; 1.0075x over previous
"""Trainium2 Bass kernel for the GRU decoder (teacher-forced) + output
projection + log_softmax.

Problem shapes: B=32, T=82, H=128, V=32000.
  log_probs [B, T, V] f32  (~336 MB -> memory-bound on the output write)
  hidden    [1, B, H] f32

Sharding: data-parallel over batch across 8 cores (4 sequences/core).
Each core:
  - gathers+relus its token embeddings (indirect DMA from emb table)
  - x_proj = x @ W_ih.T (+biases folded) as [H,3,R] tiles (bf16 matmuls)
  - runs the 82-step GRU scan in [H, B_loc] orientation; sigmoid computed
    via tanh (sigma(x) = 0.5 + 0.5*tanh(x/2)) so the ACT engine never
    switches activation tables (exp/tanh live in one table set)
  - output projection in [rows, V] orientation, 128-row tiles, 1536-wide
    vocab groups: pass 1 computes sum(exp(logits+b)) via ACT accum_out;
    log(sum) is computed with an exponent-extraction init + 2 Newton
    iterations (avoids the Ln table set); pass 2 recomputes logits and a
    single fused DVE op emits (logits - logsumexp) + b into the staging
    tile which DMAs straight out.
"""

import numpy as np
import ml_dtypes

import concourse.bacc as bacc
import concourse.bass as bass
import concourse.mybir as mybir
import concourse.tile as tile
from concourse.bass_utils import run_bass_kernel_spmd
from concourse.masks import make_identity

F32 = mybir.dt.float32
BF16 = mybir.dt.bfloat16
I32 = mybir.dt.int32
AF = mybir.ActivationFunctionType
OP = mybir.AluOpType
AX = mybir.AxisListType

B, T, H, V = 32, 82, 128, 32000
NCORES = 8
BL = B // NCORES            # 4 sequences per core
R = BL * T                  # 328 (t-major rows: r = 4*t + b)
MKS = (128, 128, 72)        # row-tile sizes
GW = 1536                   # vocab group width (3 PSUM banks)
GROUPS = [(v0, min(GW, V - v0)) for v0 in range(0, V, GW)]
LN2 = 0.6931471805599453

_NC = None


def _emit(nc, tc, ctx, d):
    consts = ctx.enter_context(tc.tile_pool(name="consts", bufs=1))
    work = ctx.enter_context(tc.tile_pool(name="work", bufs=2))
    stage_p = ctx.enter_context(tc.tile_pool(name="stagep", bufs=3))
    ps_big = ctx.enter_context(tc.tile_pool(name="psbig", bufs=2, space="PSUM"))
    ps_sm = ctx.enter_context(tc.tile_pool(name="pssm", bufs=2, space="PSUM"))

    # ---- constants into SBUF ----
    whh = consts.tile([H, 3 * H], BF16, tag="whh")
    nc.sync.dma_start(out=whh[:, :], in_=d["whht"][:, :])
    wih = consts.tile([H, 3 * H], BF16, tag="wih")
    nc.sync.dma_start(out=wih[:, :], in_=d["wiht"][:, :])
    bsc = consts.tile([H, 4], F32, tag="bsc")
    nc.sync.dma_start(out=bsc[:, :], in_=d["bsc"][:, :])
    wo = consts.tile([H, V], BF16, tag="wo")
    for i in range(8):
        s = slice(i * (V // 8), (i + 1) * (V // 8))
        nc.sync.dma_start(out=wo[:, s], in_=d["wot"][:, s])
    # bias broadcast [H, V]: load one row then doubling copies (on-chip)
    bb = consts.tile([H, V], BF16, tag="bb")
    nc.sync.dma_start(out=bb[0:1, :], in_=d["bo"][0:1, :])
    p = 1
    while p < H:
        nc.sync.dma_start(out=bb[p:2 * p, :], in_=bb[0:p, :])
        p *= 2

    h0f = consts.tile([H, BL], F32, tag="h0f")
    nc.sync.dma_start(out=h0f[:, :], in_=d["h0t"][:, :])
    h0b = consts.tile([H, BL], BF16, tag="h0b")
    nc.vector.tensor_copy(out=h0b[:, :], in_=h0f[:, :])

    ones_bf = consts.tile([1, H], BF16, tag="ones")
    nc.vector.memset(ones_bf[:, :], 1.0)
    idb = consts.tile([H, H], BF16, tag="idb")
    make_identity(nc, idb[:, :])

    # ---- embedding gather + relu + transpose ----
    xT = consts.tile([H, R], BF16, tag="xT")   # x^T, relu'd, bf16
    for k in range(3):
        mk = MKS[k]
        base = 128 * k
        tokt = consts.tile([128, 1], I32, tag=f"tok{k}")
        nc.sync.dma_start(out=tokt[:mk, 0:1], in_=d["tok"][base:base + mk])
        xf = work.tile([128, H], F32, tag="xf")
        nc.gpsimd.indirect_dma_start(
            out=xf[:mk, :],
            out_offset=None,
            in_=d["emb"][:, :],
            in_offset=bass.IndirectOffsetOnAxis(ap=tokt[:mk, 0:1], axis=0),
        )
        xb = work.tile([128, H], BF16, tag="xb")
        nc.vector.tensor_scalar_max(out=xb[:mk, :], in0=xf[:mk, :], scalar1=0.0)
        pst = ps_sm.tile([128, 384], BF16, tag="ps_sm")
        nc.tensor.transpose(out=pst[:H, :mk], in_=xb[:mk, :H], identity=idb[:mk, :mk])
        nc.vector.tensor_copy(out=xT[:, base:base + mk], in_=pst[:H, :mk])

    # ---- x_proj: xpT[:, g, :] = W_ih_g @ x^T + bias_g ----
    # bsc columns: 0: b_ih_r + b_hh_r, 1: b_ih_z + b_hh_z, 2: b_ih_n,
    #              3: 0.5 * b_hh_n
    xpT_b = consts.tile([H, 3, R], BF16, tag="xpT")
    for g in range(3):
        psx = ps_sm.tile([128, 384], F32, tag="ps_sm")
        nc.tensor.matmul(psx[:H, :R], lhsT=wih[:, g * H:(g + 1) * H],
                         rhs=xT[:, :R], start=True, stop=True)
        nc.scalar.activation(out=xpT_b[:, g, :], in_=psx[:H, :R],
                             func=AF.Identity, bias=bsc[:, g:g + 1], scale=1.0)

    # ---- GRU scan, 82 steps; h stored [H, BL] bf16 in hbT ----
    # hbT[k] col 4*j+b = (h after step t=32k+j) for batch lane b, bf16
    hbT = [consts.tile([H, 128], BF16, tag=f"hbT{k}", name=f"hbT{k}")
           for k in range(3)]
    for t in range(T):
        if t == 0:
            prev_b = h0b[:, 0:BL]
        else:
            pk, pj = (t - 1) // 32, (t - 1) % 32
            prev_b = hbT[pk][:, 4 * pj:4 * pj + 4]
        ps = ps_sm.tile([128, 384], F32, tag="ps_sm")
        # n-gate matmul first: its consumer chain is the critical path
        nc.tensor.matmul(ps[:H, 8:12], lhsT=whh[:, 2 * H:3 * H], rhs=prev_b,
                         start=True, stop=True)
        nc.tensor.matmul(ps[:H, 0:4], lhsT=whh[:, 0:H], rhs=prev_b,
                         start=True, stop=False)
        nc.tensor.matmul(ps[:H, 4:8], lhsT=whh[:, H:2 * H], rhs=prev_b,
                         start=True, stop=False)
        # accumulate xp_rz into psum so ACT reads PSUM directly
        nc.tensor.matmul(ps[:H, 0:8].rearrange("p (g b) -> p g b", g=2),
                         lhsT=idb[:, :], rhs=xpT_b[:, 0:2, 4 * t:4 * t + 4],
                         start=False, stop=True)
        # hpn_half = 0.5*(hp_n + b_hh_n)
        hpnh = work.tile([H, 4], F32, tag="hpnh")
        nc.scalar.activation(out=hpnh[:, :], in_=ps[:H, 8:12],
                             func=AF.Identity, bias=bsc[:, 3:4], scale=0.5)
        # tau = tanh(pre/2); sigma(pre) = 0.5 + 0.5*tau
        tau = work.tile([H, 8], F32, tag="tau")
        nc.scalar.activation(out=tau[:, :], in_=ps[:H, 0:8], func=AF.Tanh,
                             scale=0.5)
        # r*(hp_n + b_hh_n) = (tau_r + 1) * hpn_half
        rhpn = work.tile([H, 4], F32, tag="rhpn")
        nc.vector.scalar_tensor_tensor(out=rhpn[:, :], in0=tau[:, 0:4],
                                       scalar=1.0, in1=hpnh[:, :],
                                       op0=OP.add, op1=OP.mult)
        npre = work.tile([H, 4], F32, tag="npre")
        nc.vector.tensor_add(out=npre[:, :], in0=rhpn[:, :],
                             in1=xpT_b[:, 2, 4 * t:4 * t + 4])
        nn = work.tile([H, 4], F32, tag="nn")
        nc.scalar.activation(out=nn[:, :], in_=npre[:, :], func=AF.Tanh)
        # h' = n + z*(h - n),  z*(h-n) = 0.5*(tau_z + 1)*(h - n)
        dd = work.tile([H, 4], F32, tag="dd")
        nc.vector.tensor_sub(out=dd[:, :], in0=prev_b, in1=nn[:, :])
        ee = work.tile([H, 4], F32, tag="ee")
        nc.vector.scalar_tensor_tensor(out=ee[:, :], in0=tau[:, 4:8],
                                       scalar=1.0, in1=dd[:, :],
                                       op0=OP.add, op1=OP.mult)
        k, j = t // 32, t % 32
        nc.vector.scalar_tensor_tensor(out=hbT[k][:, 4 * j:4 * j + 4],
                                       in0=ee[:, :], scalar=0.5,
                                       in1=nn[:, :], op0=OP.mult, op1=OP.add)

    # ---- output projection + log_softmax, per 128-row tile ----
    for k in range(3):
        mk = MKS[k]
        base = 128 * k
        lhsT = hbT[k][:, 0:mk]
        acc = consts.tile([128, len(GROUPS)], F32, tag=f"acc{k}")
        # pass 1: sumexp(logits + b) per row
        for gi, (v0, ng) in enumerate(GROUPS):
            ps = ps_big.tile([128, GW], F32, tag="proj")
            for o in range(0, ng, 512):
                n = min(512, ng - o)
                nc.tensor.matmul(ps[:mk, o:o + n], lhsT=lhsT,
                                 rhs=wo[:, v0 + o:v0 + o + n],
                                 start=True, stop=False)
                nc.tensor.matmul(ps[:mk, o:o + n], lhsT=ones_bf[0:1, 0:mk],
                                 rhs=bb[0:1, v0 + o:v0 + o + n],
                                 start=False, stop=True)
            scr = work.tile([128, GW], BF16, tag="scr")
            nc.scalar.activation(out=scr[:mk, 0:ng], in_=ps[:mk, 0:ng],
                                 func=AF.Exp, accum_out=acc[:mk, gi:gi + 1])
        # logsumexp: y = log(S) via exponent-extraction init + 2 Newton steps
        S = consts.tile([128, 1], F32, tag=f"S{k}")
        nc.vector.reduce_sum(out=S[:mk, 0:1], in_=acc[:mk, 0:len(GROUPS)],
                             axis=AX.X)
        y = consts.tile([128, 1], F32, tag=f"y{k}")
        nc.vector.tensor_copy(out=y[:mk, 0:1], in_=S[:mk, 0:1].bitcast(I32))
        nc.vector.tensor_scalar(out=y[:mk, 0:1], in0=y[:mk, 0:1],
                                scalar1=LN2 / (1 << 23),
                                scalar2=-127.0 * LN2,
                                op0=OP.mult, op1=OP.add)
        em = consts.tile([128, 1], F32, tag=f"em{k}")
        for _ in range(2):
            nc.scalar.activation(out=em[:mk, 0:1], in_=y[:mk, 0:1],
                                 func=AF.Exp, scale=-1.0)
            nc.vector.scalar_tensor_tensor(out=y[:mk, 0:1], in0=em[:mk, 0:1],
                                           scalar=S[:mk, 0:1], in1=y[:mk, 0:1],
                                           op0=OP.mult, op1=OP.add)
            nc.vector.tensor_scalar_add(out=y[:mk, 0:1], in0=y[:mk, 0:1],
                                        scalar1=-1.0)
        # pass 2: out = (logits - logsumexp) + b
        for (v0, ng) in GROUPS:
            ps2 = ps_big.tile([128, GW], F32, tag="proj")
            for o in range(0, ng, 512):
                n = min(512, ng - o)
                nc.tensor.matmul(ps2[:mk, o:o + n], lhsT=lhsT,
                                 rhs=wo[:, v0 + o:v0 + o + n],
                                 start=True, stop=True)
            st = stage_p.tile([128, GW], F32, tag="stage")
            nc.vector.scalar_tensor_tensor(out=st[:mk, 0:ng],
                                           in0=ps2[:mk, 0:ng],
                                           scalar=y[:mk, 0:1],
                                           in1=bb[:mk, v0:v0 + ng],
                                           op0=OP.subtract, op1=OP.add)
            nc.sync.dma_start(out=d["olp"][base:base + mk, v0:v0 + ng],
                              in_=st[:mk, 0:ng])

    # ---- final hidden state (bf16 in hbT[2] cols 68:72) -> [BL, H] ----
    pso = ps_sm.tile([128, 384], BF16, tag="ps_sm")
    nc.tensor.transpose(out=pso[0:BL, 0:H], in_=hbT[2][:, 68:72],
                        identity=idb[:, :])
    oh_sb = consts.tile([BL, H], F32, tag="oh")
    nc.vector.tensor_copy(out=oh_sb[:, :], in_=pso[0:BL, 0:H])
    nc.sync.dma_start(out=d["oh"][:, :], in_=oh_sb[:, :])


def _build():
    from contextlib import ExitStack
    nc = bacc.Bacc(None, target_bir_lowering=False)
    d = {
        "tok": nc.dram_tensor("tok", [R], I32, kind="ExternalInput"),
        "emb": nc.dram_tensor("emb", [V, H], F32, kind="ExternalInput"),
        "h0t": nc.dram_tensor("h0t", [H, BL], F32, kind="ExternalInput"),
        "wiht": nc.dram_tensor("wiht", [H, 3 * H], BF16, kind="ExternalInput"),
        "whht": nc.dram_tensor("whht", [H, 3 * H], BF16, kind="ExternalInput"),
        "bsc": nc.dram_tensor("bsc", [H, 4], F32, kind="ExternalInput"),
        "wot": nc.dram_tensor("wot", [H, V], BF16, kind="ExternalInput"),
        "bo": nc.dram_tensor("bo", [1, V], BF16, kind="ExternalInput"),
        "olp": nc.dram_tensor("olp", [R, V], F32, kind="ExternalOutput"),
        "oh": nc.dram_tensor("oh", [BL, H], F32, kind="ExternalOutput"),
    }
    with tile.TileContext(nc) as tc:
        with ExitStack() as ctx:
            _emit(nc, tc, ctx, d)
    nc.finalize()
    return nc


def _prepare_in_maps(encoder_hidden, target_tensor, emb,
                     W_ih, W_hh, b_ih, b_hh, W_out, b_out):
    bf16 = ml_dtypes.bfloat16
    tgt = np.asarray(target_tensor)
    tokens = np.concatenate(
        [np.zeros((B, 1), tgt.dtype), tgt[:, :-1]], axis=1).astype(np.int32)
    ehid = np.asarray(encoder_hidden, np.float32)
    embf = np.ascontiguousarray(np.asarray(emb, np.float32))
    b_ih = np.asarray(b_ih, np.float32)
    b_hh = np.asarray(b_hh, np.float32)
    wihT = np.ascontiguousarray(np.asarray(W_ih, np.float32).T).astype(bf16)
    whhT = np.ascontiguousarray(np.asarray(W_hh, np.float32).T).astype(bf16)
    woT = np.ascontiguousarray(np.asarray(W_out, np.float32).T).astype(bf16)
    bo = np.asarray(b_out, np.float32)[None, :].astype(bf16)
    bsc = np.ascontiguousarray(np.stack(
        [b_ih[0:H] + b_hh[0:H],
         b_ih[H:2 * H] + b_hh[H:2 * H],
         b_ih[2 * H:3 * H],
         0.5 * b_hh[2 * H:3 * H]], axis=1).astype(np.float32))

    in_maps = []
    for c in range(NCORES):
        bs = slice(BL * c, BL * (c + 1))
        tok_c = np.ascontiguousarray(tokens[bs].T).reshape(-1)  # t-major [R]
        h0T_c = np.ascontiguousarray(ehid[0, bs, :].T)          # [H, BL]
        in_maps.append({
            "tok": tok_c, "emb": embf, "h0t": h0T_c, "wiht": wihT,
            "whht": whhT, "bsc": bsc, "wot": woT, "bo": bo,
        })
    return in_maps


def _assemble(results):
    lp = np.empty((B, T, V), np.float32)
    hT = np.empty((1, B, H), np.float32)
    for c in range(NCORES):
        r = results[c]
        lp[BL * c:BL * (c + 1)] = \
            r["olp"].reshape(T, BL, V).transpose(1, 0, 2)
        hT[0, BL * c:BL * (c + 1)] = r["oh"]
    return lp, hT


def kernel(encoder_outputs, encoder_hidden, target_tensor, emb,
           W_ih, W_hh, b_ih, b_hh, W_out, b_out):
    global _NC
    if _NC is None:
        _NC = _build()
    in_maps = _prepare_in_maps(encoder_hidden, target_tensor, emb,
                               W_ih, W_hh, b_ih, b_hh, W_out, b_out)
    res = run_bass_kernel_spmd(_NC, in_maps, list(range(NCORES)))
    return _assemble(res.results)


# revision 25
# speedup vs baseline: 1.0513x; 1.0435x over previous
"""Trainium2 Bass kernel for the GRU decoder (teacher-forced) + output
projection + log_softmax.

Problem shapes: B=32, T=82, H=128, V=32000.
  log_probs [B, T, V] f32  (~336 MB -> memory-bound on the output write)
  hidden    [1, B, H] f32

Sharding: data-parallel over batch across 8 cores (4 sequences/core).
Each core:
  - gathers+relus its token embeddings (indirect DMA from emb table)
  - x_proj = x @ W_ih.T (+biases folded) as [H,3,R] tiles (bf16 matmuls)
  - runs the 82-step GRU scan in [H, B_loc] orientation; sigmoid computed
    via tanh (sigma(x) = 0.5 + 0.5*tanh(x/2)) so the ACT engine never
    switches activation tables (exp/tanh live in one table set)
  - output projection in [rows, V] orientation, 128-row tiles, 1536-wide
    vocab groups: pass 1 computes sum(exp(logits+b)) via ACT accum_out;
    log(sum) is computed with an exponent-extraction init + 2 Newton
    iterations (avoids the Ln table set); pass 2 recomputes logits and a
    single fused DVE op emits (logits - logsumexp) + b into the staging
    tile which DMAs straight out.
"""

import numpy as np
import ml_dtypes

import concourse.bacc as bacc
import concourse.bass as bass
import concourse.mybir as mybir
import concourse.tile as tile
from concourse.bass_utils import run_bass_kernel_spmd
from concourse.masks import make_identity

F32 = mybir.dt.float32
BF16 = mybir.dt.bfloat16
I32 = mybir.dt.int32
AF = mybir.ActivationFunctionType
OP = mybir.AluOpType
AX = mybir.AxisListType

B, T, H, V = 32, 82, 128, 32000
NCORES = 8
BL = B // NCORES            # 4 sequences per core
R = BL * T                  # 328 (t-major rows: r = 4*t + b)
MKS = (128, 128, 72)        # row-tile sizes
GW1 = 1024                  # pass-1 vocab group width (2 PSUM banks)
GROUPS1 = [(v0, min(GW1, V - v0)) for v0 in range(0, V, GW1)]
GW2 = 512                   # pass-2 chunk width (1 PSUM bank)
GWS = 1536                  # pass-2 staging/DMA width
LN2 = 0.6931471805599453

_NC = None


def _emit(nc, tc, ctx, d):
    consts = ctx.enter_context(tc.tile_pool(name="consts", bufs=1))
    work = ctx.enter_context(tc.tile_pool(name="work", bufs=2))
    stage_p = ctx.enter_context(tc.tile_pool(name="stagep", bufs=3))
    ps_p1 = ctx.enter_context(tc.tile_pool(name="psp1", bufs=2, space="PSUM"))
    ps_p2 = ctx.enter_context(tc.tile_pool(name="psp2", bufs=2, space="PSUM"))
    ps_sm = ctx.enter_context(tc.tile_pool(name="pssm", bufs=2, space="PSUM"))

    # ---- constants into SBUF ----
    whh = consts.tile([H, 3 * H], BF16, tag="whh")
    nc.sync.dma_start(out=whh[:, :], in_=d["whht"][:, :])
    wih = consts.tile([H, 3 * H], BF16, tag="wih")
    nc.sync.dma_start(out=wih[:, :], in_=d["wiht"][:, :])
    bsc = consts.tile([H, 4], F32, tag="bsc")
    nc.sync.dma_start(out=bsc[:, :], in_=d["bsc"][:, :])
    wo = consts.tile([H, V], BF16, tag="wo")
    for i in range(8):
        s = slice(i * (V // 8), (i + 1) * (V // 8))
        nc.sync.dma_start(out=wo[:, s], in_=d["wot"][:, s])
    # bias broadcast [H, V]: load one row then doubling copies (on-chip)
    bb = consts.tile([H, V], BF16, tag="bb")
    nc.sync.dma_start(out=bb[0:1, :], in_=d["bo"][0:1, :])
    p = 1
    while p < H:
        nc.sync.dma_start(out=bb[p:2 * p, :], in_=bb[0:p, :])
        p *= 2

    h0f = consts.tile([H, BL], F32, tag="h0f")
    nc.sync.dma_start(out=h0f[:, :], in_=d["h0t"][:, :])
    h0b = consts.tile([H, BL], BF16, tag="h0b")
    nc.vector.tensor_copy(out=h0b[:, :], in_=h0f[:, :])

    ones_bf = consts.tile([1, H], BF16, tag="ones")
    nc.vector.memset(ones_bf[:, :], 1.0)
    idb = consts.tile([H, H], BF16, tag="idb")
    make_identity(nc, idb[:, :])
    idf = consts.tile([H, H], F32, tag="idf")
    make_identity(nc, idf[:, :])

    # ---- embedding gather + relu + transpose ----
    xT = consts.tile([H, R], BF16, tag="xT")   # x^T, relu'd, bf16
    for k in range(3):
        mk = MKS[k]
        base = 128 * k
        tokt = consts.tile([128, 1], I32, tag=f"tok{k}")
        nc.sync.dma_start(out=tokt[:mk, 0:1], in_=d["tok"][base:base + mk])
        xf = work.tile([128, H], F32, tag="xf")
        nc.gpsimd.indirect_dma_start(
            out=xf[:mk, :],
            out_offset=None,
            in_=d["emb"][:, :],
            in_offset=bass.IndirectOffsetOnAxis(ap=tokt[:mk, 0:1], axis=0),
        )
        xb = work.tile([128, H], BF16, tag="xb")
        nc.vector.tensor_scalar_max(out=xb[:mk, :], in0=xf[:mk, :], scalar1=0.0)
        pst = ps_sm.tile([128, 384], BF16, tag="ps_sm")
        nc.tensor.transpose(out=pst[:H, :mk], in_=xb[:mk, :H], identity=idb[:mk, :mk])
        nc.vector.tensor_copy(out=xT[:, base:base + mk], in_=pst[:H, :mk])

    # ---- x_proj: xpT[:, g, :] = W_ih_g @ x^T + bias_g ----
    # bsc columns: 0: b_ih_r + b_hh_r, 1: b_ih_z + b_hh_z, 2: b_ih_n,
    #              3: 0.5 * b_hh_n
    xpT = consts.tile([H, 3, R], F32, tag="xpT")
    for g in range(3):
        psx = ps_sm.tile([128, 384], F32, tag="ps_sm")
        nc.tensor.matmul(psx[:H, :R], lhsT=wih[:, g * H:(g + 1) * H],
                         rhs=xT[:, :R], start=True, stop=True)
        nc.scalar.activation(out=xpT[:, g, :], in_=psx[:H, :R],
                             func=AF.Identity, bias=bsc[:, g:g + 1], scale=1.0)

    # ---- GRU scan, 82 steps; h stored [H, BL] bf16 in hbT ----
    # hbT[k] col 4*j+b = (h after step t=32k+j) for batch lane b, bf16
    hbT = [consts.tile([H, 128], BF16, tag=f"hbT{k}", name=f"hbT{k}")
           for k in range(3)]
    hf_prev = h0f
    for t in range(T):
        if t == 0:
            prev_b = h0b[:, 0:BL]
        else:
            pk, pj = (t - 1) // 32, (t - 1) % 32
            prev_b = hbT[pk][:, 4 * pj:4 * pj + 4]
        ps = ps_sm.tile([128, 384], F32, tag="ps_sm")
        # n-gate matmul first: its consumer chain is the critical path
        nc.tensor.matmul(ps[:H, 8:12], lhsT=whh[:, 2 * H:3 * H],
                         rhs=prev_b, start=True, stop=True)
        nc.tensor.matmul(ps[:H, 0:4], lhsT=whh[:, 0:H],
                         rhs=prev_b, start=True, stop=True)
        nc.tensor.matmul(ps[:H, 4:8], lhsT=whh[:, H:2 * H],
                         rhs=prev_b, start=True, stop=True)
        # hpn_half = 0.5*(hp_n + b_hh_n)
        hpnh = work.tile([H, 4], F32, tag="hpnh")
        nc.scalar.activation(out=hpnh[:, :], in_=ps[:H, 8:12],
                             func=AF.Identity, bias=bsc[:, 3:4], scale=0.5)
        # pre_rz = hp_rz + xp_rz   (biases already folded into xpT)
        pre = work.tile([H, 8], F32, tag="pre")
        nc.vector.tensor_add(
            out=pre[:, :].rearrange("p (g b) -> p g b", g=2),
            in0=ps[:H, 0:8].rearrange("p (g b) -> p g b", g=2),
            in1=xpT[:, 0:2, 4 * t:4 * t + 4])
        # tau = tanh(pre/2); sigma(pre) = 0.5 + 0.5*tau
        tau = work.tile([H, 8], F32, tag="tau")
        nc.scalar.activation(out=tau[:, :], in_=pre[:, :], func=AF.Tanh,
                             scale=0.5)
        # r*(hp_n + b_hh_n) = (tau_r + 1) * hpn_half
        rhpn = work.tile([H, 4], F32, tag="rhpn")
        nc.vector.scalar_tensor_tensor(out=rhpn[:, :], in0=tau[:, 0:4],
                                       scalar=1.0, in1=hpnh[:, :],
                                       op0=OP.add, op1=OP.mult)
        npre = work.tile([H, 4], F32, tag="npre")
        nc.vector.tensor_add(out=npre[:, :], in0=rhpn[:, :],
                             in1=xpT[:, 2, 4 * t:4 * t + 4])
        nn = work.tile([H, 4], F32, tag="nn")
        nc.scalar.activation(out=nn[:, :], in_=npre[:, :], func=AF.Tanh)
        # h' = n + z*(h - n),  z*(h-n) = 0.5*(tau_z + 1)*(h - n)
        dd = work.tile([H, 4], F32, tag="dd")
        nc.vector.tensor_sub(out=dd[:, :], in0=hf_prev[:, 0:BL], in1=nn[:, :])
        ee = work.tile([H, 4], F32, tag="ee")
        nc.vector.scalar_tensor_tensor(out=ee[:, :], in0=tau[:, 4:8],
                                       scalar=1.0, in1=dd[:, :],
                                       op0=OP.add, op1=OP.mult)
        hf = work.tile([H, 4], F32, tag="hf")
        nc.vector.scalar_tensor_tensor(out=hf[:, :], in0=ee[:, :],
                                       scalar=0.5, in1=nn[:, :],
                                       op0=OP.mult, op1=OP.add)
        k, j = t // 32, t % 32
        nc.vector.tensor_copy(out=hbT[k][:, 4 * j:4 * j + 4], in_=hf[:, :])
        hf_prev = hf

    # ---- output projection + log_softmax, per 128-row tile ----
    for k in range(3):
        mk = MKS[k]
        base = 128 * k
        lhsT = hbT[k][:, 0:mk]
        acc = consts.tile([128, len(GROUPS1)], F32, tag=f"acc{k}")
        # pass 1: sumexp(logits + b) per row
        for gi, (v0, ng) in enumerate(GROUPS1):
            ps = ps_p1.tile([128, GW1], F32, tag="p1")
            for o in range(0, ng, 512):
                n = min(512, ng - o)
                nc.tensor.matmul(ps[:mk, o:o + n], lhsT=lhsT,
                                 rhs=wo[:, v0 + o:v0 + o + n],
                                 start=True, stop=False)
                nc.tensor.matmul(ps[:mk, o:o + n], lhsT=ones_bf[0:1, 0:mk],
                                 rhs=bb[0:1, v0 + o:v0 + o + n],
                                 start=False, stop=True)
            scr = work.tile([128, GW1], BF16, tag="scr")
            nc.scalar.activation(out=scr[:mk, 0:ng], in_=ps[:mk, 0:ng],
                                 func=AF.Exp, accum_out=acc[:mk, gi:gi + 1])
        # logsumexp: y = log(S) via exponent-extraction init + 2 Newton steps
        S = consts.tile([128, 1], F32, tag=f"S{k}")
        nc.vector.reduce_sum(out=S[:mk, 0:1], in_=acc[:mk, 0:len(GROUPS1)],
                             axis=AX.X)
        y = consts.tile([128, 1], F32, tag=f"y{k}")
        nc.vector.tensor_copy(out=y[:mk, 0:1], in_=S[:mk, 0:1].bitcast(I32))
        nc.vector.tensor_scalar(out=y[:mk, 0:1], in0=y[:mk, 0:1],
                                scalar1=LN2 / (1 << 23),
                                scalar2=-127.0 * LN2,
                                op0=OP.mult, op1=OP.add)
        em = consts.tile([128, 1], F32, tag=f"em{k}")
        for _ in range(2):
            nc.scalar.activation(out=em[:mk, 0:1], in_=y[:mk, 0:1],
                                 func=AF.Exp, scale=-1.0)
            nc.vector.scalar_tensor_tensor(out=y[:mk, 0:1], in0=em[:mk, 0:1],
                                           scalar=S[:mk, 0:1], in1=y[:mk, 0:1],
                                           op0=OP.mult, op1=OP.add)
            nc.vector.tensor_scalar_add(out=y[:mk, 0:1], in0=y[:mk, 0:1],
                                        scalar1=-1.0)
        # pass 2: out = (logits - logsumexp) + b
        for s0 in range(0, V, GWS):
            ns = min(GWS, V - s0)
            st = stage_p.tile([128, GWS], F32, tag="stage")
            for o in range(0, ns, GW2):
                n = min(GW2, ns - o)
                v0 = s0 + o
                ps2 = ps_p2.tile([128, GW2], F32, tag="p2")
                nc.tensor.matmul(ps2[:mk, 0:n], lhsT=lhsT,
                                 rhs=wo[:, v0:v0 + n],
                                 start=True, stop=True)
                nc.vector.scalar_tensor_tensor(out=st[:mk, o:o + n],
                                               in0=ps2[:mk, 0:n],
                                               scalar=y[:mk, 0:1],
                                               in1=bb[:mk, v0:v0 + n],
                                               op0=OP.subtract, op1=OP.add)
            nc.sync.dma_start(out=d["olp"][base:base + mk, s0:s0 + ns],
                              in_=st[:mk, 0:ns])

    # ---- final hidden state -> [BL, H] ----
    pso = ps_sm.tile([128, 384], F32, tag="ps_sm")
    nc.tensor.transpose(out=pso[0:BL, 0:H], in_=hf_prev[:, 0:BL],
                        identity=idf[:, :])
    oh_sb = consts.tile([BL, H], F32, tag="oh")
    nc.vector.tensor_copy(out=oh_sb[:, :], in_=pso[0:BL, 0:H])
    nc.sync.dma_start(out=d["oh"][:, :], in_=oh_sb[:, :])


def _build():
    from contextlib import ExitStack
    nc = bacc.Bacc(None, target_bir_lowering=False)
    d = {
        "tok": nc.dram_tensor("tok", [R], I32, kind="ExternalInput"),
        "emb": nc.dram_tensor("emb", [V, H], F32, kind="ExternalInput"),
        "h0t": nc.dram_tensor("h0t", [H, BL], F32, kind="ExternalInput"),
        "wiht": nc.dram_tensor("wiht", [H, 3 * H], BF16, kind="ExternalInput"),
        "whht": nc.dram_tensor("whht", [H, 3 * H], BF16, kind="ExternalInput"),
        "bsc": nc.dram_tensor("bsc", [H, 4], F32, kind="ExternalInput"),
        "wot": nc.dram_tensor("wot", [H, V], BF16, kind="ExternalInput"),
        "bo": nc.dram_tensor("bo", [1, V], BF16, kind="ExternalInput"),
        "olp": nc.dram_tensor("olp", [R, V], F32, kind="ExternalOutput"),
        "oh": nc.dram_tensor("oh", [BL, H], F32, kind="ExternalOutput"),
    }
    with tile.TileContext(nc) as tc:
        with ExitStack() as ctx:
            _emit(nc, tc, ctx, d)
    nc.finalize()
    return nc


def _prepare_in_maps(encoder_hidden, target_tensor, emb,
                     W_ih, W_hh, b_ih, b_hh, W_out, b_out):
    bf16 = ml_dtypes.bfloat16
    tgt = np.asarray(target_tensor)
    tokens = np.concatenate(
        [np.zeros((B, 1), tgt.dtype), tgt[:, :-1]], axis=1).astype(np.int32)
    ehid = np.asarray(encoder_hidden, np.float32)
    embf = np.ascontiguousarray(np.asarray(emb, np.float32))
    b_ih = np.asarray(b_ih, np.float32)
    b_hh = np.asarray(b_hh, np.float32)
    wihT = np.ascontiguousarray(np.asarray(W_ih, np.float32).T).astype(bf16)
    whhT = np.ascontiguousarray(np.asarray(W_hh, np.float32).T).astype(bf16)
    woT = np.ascontiguousarray(np.asarray(W_out, np.float32).T).astype(bf16)
    bo = np.asarray(b_out, np.float32)[None, :].astype(bf16)
    bsc = np.ascontiguousarray(np.stack(
        [b_ih[0:H] + b_hh[0:H],
         b_ih[H:2 * H] + b_hh[H:2 * H],
         b_ih[2 * H:3 * H],
         0.5 * b_hh[2 * H:3 * H]], axis=1).astype(np.float32))

    in_maps = []
    for c in range(NCORES):
        bs = slice(BL * c, BL * (c + 1))
        tok_c = np.ascontiguousarray(tokens[bs].T).reshape(-1)  # t-major [R]
        h0T_c = np.ascontiguousarray(ehid[0, bs, :].T)          # [H, BL]
        in_maps.append({
            "tok": tok_c, "emb": embf, "h0t": h0T_c, "wiht": wihT,
            "whht": whhT, "bsc": bsc, "wot": woT, "bo": bo,
        })
    return in_maps


def _assemble(results):
    lp = np.empty((B, T, V), np.float32)
    hT = np.empty((1, B, H), np.float32)
    for c in range(NCORES):
        r = results[c]
        lp[BL * c:BL * (c + 1)] = \
            r["olp"].reshape(T, BL, V).transpose(1, 0, 2)
        hT[0, BL * c:BL * (c + 1)] = r["oh"]
    return lp, hT


def kernel(encoder_outputs, encoder_hidden, target_tensor, emb,
           W_ih, W_hh, b_ih, b_hh, W_out, b_out):
    global _NC
    if _NC is None:
        _NC = _build()
    in_maps = _prepare_in_maps(encoder_hidden, target_tensor, emb,
                               W_ih, W_hh, b_ih, b_hh, W_out, b_out)
    res = run_bass_kernel_spmd(_NC, in_maps, list(range(NCORES)))
    return _assemble(res.results)


# revision 27
# speedup vs baseline: 1.0630x; 1.0112x over previous
"""Trainium2 Bass kernel for the GRU decoder (teacher-forced) + output
projection + log_softmax.

Problem shapes: B=32, T=82, H=128, V=32000.
  log_probs [B, T, V] f32  (~336 MB -> memory-bound on the output write)
  hidden    [1, B, H] f32

Sharding: data-parallel over batch across 8 cores (4 sequences/core).
Each core:
  - gathers+relus its token embeddings (indirect DMA from emb table)
  - x_proj = x @ W_ih.T (+biases folded) as [H,3,R] tiles (bf16 matmuls)
  - runs the 82-step GRU scan in [H, B_loc] orientation; sigmoid computed
    via tanh (sigma(x) = 0.5 + 0.5*tanh(x/2)) so the ACT engine never
    switches activation tables (exp/tanh live in one table set)
  - output projection in [rows, V] orientation, 128-row tiles, 1536-wide
    vocab groups: pass 1 computes sum(exp(logits+b)) via ACT accum_out;
    log(sum) is computed with an exponent-extraction init + 2 Newton
    iterations (avoids the Ln table set); pass 2 recomputes logits and a
    single fused DVE op emits (logits - logsumexp) + b into the staging
    tile which DMAs straight out.
"""

import numpy as np
import ml_dtypes

import concourse.bacc as bacc
import concourse.bass as bass
import concourse.mybir as mybir
import concourse.tile as tile
from concourse.bass_utils import run_bass_kernel_spmd
from concourse.masks import make_identity

F32 = mybir.dt.float32
BF16 = mybir.dt.bfloat16
I32 = mybir.dt.int32
AF = mybir.ActivationFunctionType
OP = mybir.AluOpType
AX = mybir.AxisListType

B, T, H, V = 32, 82, 128, 32000
NCORES = 8
BL = B // NCORES            # 4 sequences per core
R = BL * T                  # 328 (t-major rows: r = 4*t + b)
MKS = (128, 128, 72)        # row-tile sizes
GW1 = 1024                  # pass-1 vocab group width (2 PSUM banks)
GROUPS1 = [(v0, min(GW1, V - v0)) for v0 in range(0, V, GW1)]
GW2 = 512                   # pass-2 chunk width (1 PSUM bank)
GWS = 1536                  # pass-2 staging/DMA width
LN2 = 0.6931471805599453

_NC = None


def _emit(nc, tc, ctx, d):
    consts = ctx.enter_context(tc.tile_pool(name="consts", bufs=1))
    work = ctx.enter_context(tc.tile_pool(name="work", bufs=3))
    stage_p = ctx.enter_context(tc.tile_pool(name="stagep", bufs=4))
    ps_p1 = ctx.enter_context(tc.tile_pool(name="psp1", bufs=2, space="PSUM"))
    ps_p2 = ctx.enter_context(tc.tile_pool(name="psp2", bufs=2, space="PSUM"))
    ps_sm = ctx.enter_context(tc.tile_pool(name="pssm", bufs=2, space="PSUM"))

    # ---- constants into SBUF ----
    whh = consts.tile([H, 3 * H], BF16, tag="whh")
    nc.sync.dma_start(out=whh[:, :], in_=d["whht"][:, :])
    wih = consts.tile([H, 3 * H], BF16, tag="wih")
    nc.sync.dma_start(out=wih[:, :], in_=d["wiht"][:, :])
    bsc = consts.tile([H, 4], F32, tag="bsc")
    nc.sync.dma_start(out=bsc[:, :], in_=d["bsc"][:, :])
    wo = consts.tile([H, V], BF16, tag="wo")
    for i in range(8):
        s = slice(i * (V // 8), (i + 1) * (V // 8))
        nc.sync.dma_start(out=wo[:, s], in_=d["wot"][:, s])
    # bias broadcast [H, V]: load one row then doubling copies (on-chip)
    bb = consts.tile([H, V], BF16, tag="bb")
    nc.sync.dma_start(out=bb[0:1, :], in_=d["bo"][0:1, :])
    p = 1
    while p < H:
        nc.sync.dma_start(out=bb[p:2 * p, :], in_=bb[0:p, :])
        p *= 2

    h0f = consts.tile([H, BL], F32, tag="h0f")
    nc.sync.dma_start(out=h0f[:, :], in_=d["h0t"][:, :])
    h0b = consts.tile([H, BL], BF16, tag="h0b")
    nc.vector.tensor_copy(out=h0b[:, :], in_=h0f[:, :])

    ones_bf = consts.tile([1, H], BF16, tag="ones")
    nc.vector.memset(ones_bf[:, :], 1.0)
    idb = consts.tile([H, H], BF16, tag="idb")
    make_identity(nc, idb[:, :])
    idf = consts.tile([H, H], F32, tag="idf")
    make_identity(nc, idf[:, :])

    # ---- embedding gather + relu + transpose ----
    xT = consts.tile([H, R], BF16, tag="xT")   # x^T, relu'd, bf16
    for k in range(3):
        mk = MKS[k]
        base = 128 * k
        tokt = consts.tile([128, 1], I32, tag=f"tok{k}")
        nc.sync.dma_start(out=tokt[:mk, 0:1], in_=d["tok"][base:base + mk])
        xf = work.tile([128, H], F32, tag="xf")
        nc.gpsimd.indirect_dma_start(
            out=xf[:mk, :],
            out_offset=None,
            in_=d["emb"][:, :],
            in_offset=bass.IndirectOffsetOnAxis(ap=tokt[:mk, 0:1], axis=0),
        )
        xb = work.tile([128, H], BF16, tag="xb")
        nc.vector.tensor_scalar_max(out=xb[:mk, :], in0=xf[:mk, :], scalar1=0.0)
        pst = ps_sm.tile([128, 384], BF16, tag="ps_sm")
        nc.tensor.transpose(out=pst[:H, :mk], in_=xb[:mk, :H], identity=idb[:mk, :mk])
        nc.vector.tensor_copy(out=xT[:, base:base + mk], in_=pst[:H, :mk])

    # ---- x_proj: xpT[:, g, :] = W_ih_g @ x^T + bias_g ----
    # bsc columns: 0: b_ih_r + b_hh_r, 1: b_ih_z + b_hh_z, 2: b_ih_n,
    #              3: 0.5 * b_hh_n
    xpT = consts.tile([H, 3, R], F32, tag="xpT")
    for g in range(3):
        psx = ps_sm.tile([128, 384], F32, tag="ps_sm")
        nc.tensor.matmul(psx[:H, :R], lhsT=wih[:, g * H:(g + 1) * H],
                         rhs=xT[:, :R], start=True, stop=True)
        nc.scalar.activation(out=xpT[:, g, :], in_=psx[:H, :R],
                             func=AF.Identity, bias=bsc[:, g:g + 1], scale=1.0)

    # ---- GRU scan, 82 steps; h stored [H, BL] bf16 in hbT ----
    # hbT[k] col 4*j+b = (h after step t=32k+j) for batch lane b, bf16
    hbT = [consts.tile([H, 128], BF16, tag=f"hbT{k}", name=f"hbT{k}")
           for k in range(3)]
    hf_prev = h0f
    for t in range(T):
        if t == 0:
            prev_b = h0b[:, 0:BL]
        else:
            pk, pj = (t - 1) // 32, (t - 1) % 32
            prev_b = hbT[pk][:, 4 * pj:4 * pj + 4]
        ps = ps_sm.tile([128, 384], F32, tag="ps_sm")
        # n-gate matmul first: its consumer chain is the critical path
        nc.tensor.matmul(ps[:H, 8:12], lhsT=whh[:, 2 * H:3 * H],
                         rhs=prev_b, start=True, stop=True)
        nc.tensor.matmul(ps[:H, 0:4], lhsT=whh[:, 0:H],
                         rhs=prev_b, start=True, stop=True)
        nc.tensor.matmul(ps[:H, 4:8], lhsT=whh[:, H:2 * H],
                         rhs=prev_b, start=True, stop=True)
        # hpn_half = 0.5*(hp_n + b_hh_n)
        hpnh = work.tile([H, 4], F32, tag="hpnh")
        nc.scalar.activation(out=hpnh[:, :], in_=ps[:H, 8:12],
                             func=AF.Identity, bias=bsc[:, 3:4], scale=0.5)
        # pre_rz = hp_rz + xp_rz   (biases already folded into xpT)
        pre = work.tile([H, 8], F32, tag="pre")
        nc.vector.tensor_add(
            out=pre[:, :].rearrange("p (g b) -> p g b", g=2),
            in0=ps[:H, 0:8].rearrange("p (g b) -> p g b", g=2),
            in1=xpT[:, 0:2, 4 * t:4 * t + 4])
        # tau = tanh(pre/2); sigma(pre) = 0.5 + 0.5*tau
        tau = work.tile([H, 8], F32, tag="tau")
        nc.scalar.activation(out=tau[:, :], in_=pre[:, :], func=AF.Tanh,
                             scale=0.5)
        # r*(hp_n + b_hh_n) = (tau_r + 1) * hpn_half
        rhpn = work.tile([H, 4], F32, tag="rhpn")
        nc.vector.scalar_tensor_tensor(out=rhpn[:, :], in0=tau[:, 0:4],
                                       scalar=1.0, in1=hpnh[:, :],
                                       op0=OP.add, op1=OP.mult)
        npre = work.tile([H, 4], F32, tag="npre")
        nc.vector.tensor_add(out=npre[:, :], in0=rhpn[:, :],
                             in1=xpT[:, 2, 4 * t:4 * t + 4])
        nn = work.tile([H, 4], F32, tag="nn")
        nc.scalar.activation(out=nn[:, :], in_=npre[:, :], func=AF.Tanh)
        # h' = n + z*(h - n),  z*(h-n) = 0.5*(tau_z + 1)*(h - n)
        dd = work.tile([H, 4], F32, tag="dd")
        nc.vector.tensor_sub(out=dd[:, :], in0=hf_prev[:, 0:BL], in1=nn[:, :])
        ee = work.tile([H, 4], F32, tag="ee")
        nc.vector.scalar_tensor_tensor(out=ee[:, :], in0=tau[:, 4:8],
                                       scalar=1.0, in1=dd[:, :],
                                       op0=OP.add, op1=OP.mult)
        hf = work.tile([H, 4], F32, tag="hf")
        nc.vector.scalar_tensor_tensor(out=hf[:, :], in0=ee[:, :],
                                       scalar=0.5, in1=nn[:, :],
                                       op0=OP.mult, op1=OP.add)
        k, j = t // 32, t % 32
        nc.gpsimd.tensor_copy(out=hbT[k][:, 4 * j:4 * j + 4], in_=hf[:, :])
        hf_prev = hf

    # ---- output projection + log_softmax, per 128-row tile ----
    for k in range(3):
        mk = MKS[k]
        base = 128 * k
        lhsT = hbT[k][:, 0:mk]
        acc = consts.tile([128, len(GROUPS1)], F32, tag=f"acc{k}")
        # pass 1: sumexp(logits + b) per row
        for gi, (v0, ng) in enumerate(GROUPS1):
            ps = ps_p1.tile([128, GW1], F32, tag="p1")
            for o in range(0, ng, 512):
                n = min(512, ng - o)
                nc.tensor.matmul(ps[:mk, o:o + n], lhsT=lhsT,
                                 rhs=wo[:, v0 + o:v0 + o + n],
                                 start=True, stop=False)
                nc.tensor.matmul(ps[:mk, o:o + n], lhsT=ones_bf[0:1, 0:mk],
                                 rhs=bb[0:1, v0 + o:v0 + o + n],
                                 start=False, stop=True)
            scr = work.tile([128, GW1], BF16, tag="scr")
            nc.scalar.activation(out=scr[:mk, 0:ng], in_=ps[:mk, 0:ng],
                                 func=AF.Exp, accum_out=acc[:mk, gi:gi + 1])
        # logsumexp: y = log(S) via exponent-extraction init + 2 Newton steps
        S = consts.tile([128, 1], F32, tag=f"S{k}")
        nc.vector.reduce_sum(out=S[:mk, 0:1], in_=acc[:mk, 0:len(GROUPS1)],
                             axis=AX.X)
        y = consts.tile([128, 1], F32, tag=f"y{k}")
        nc.vector.tensor_copy(out=y[:mk, 0:1], in_=S[:mk, 0:1].bitcast(I32))
        nc.vector.tensor_scalar(out=y[:mk, 0:1], in0=y[:mk, 0:1],
                                scalar1=LN2 / (1 << 23),
                                scalar2=-127.0 * LN2,
                                op0=OP.mult, op1=OP.add)
        em = consts.tile([128, 1], F32, tag=f"em{k}")
        for _ in range(2):
            nc.scalar.activation(out=em[:mk, 0:1], in_=y[:mk, 0:1],
                                 func=AF.Exp, scale=-1.0)
            nc.vector.scalar_tensor_tensor(out=y[:mk, 0:1], in0=em[:mk, 0:1],
                                           scalar=S[:mk, 0:1], in1=y[:mk, 0:1],
                                           op0=OP.mult, op1=OP.add)
            nc.vector.tensor_scalar_add(out=y[:mk, 0:1], in0=y[:mk, 0:1],
                                        scalar1=-1.0)
        # pass 2: out = (logits - logsumexp) + b
        for s0 in range(0, V, GWS):
            ns = min(GWS, V - s0)
            st = stage_p.tile([128, GWS], F32, tag="stage")
            for o in range(0, ns, GW2):
                n = min(GW2, ns - o)
                v0 = s0 + o
                ps2 = ps_p2.tile([128, GW2], F32, tag="p2")
                nc.tensor.matmul(ps2[:mk, 0:n], lhsT=lhsT,
                                 rhs=wo[:, v0:v0 + n],
                                 start=True, stop=True)
                nc.vector.scalar_tensor_tensor(out=st[:mk, o:o + n],
                                               in0=ps2[:mk, 0:n],
                                               scalar=y[:mk, 0:1],
                                               in1=bb[:mk, v0:v0 + n],
                                               op0=OP.subtract, op1=OP.add)
            nc.sync.dma_start(out=d["olp"][base:base + mk, s0:s0 + ns],
                              in_=st[:mk, 0:ns])

    # ---- final hidden state -> [BL, H] ----
    pso = ps_sm.tile([128, 384], F32, tag="ps_sm")
    nc.tensor.transpose(out=pso[0:BL, 0:H], in_=hf_prev[:, 0:BL],
                        identity=idf[:, :])
    oh_sb = consts.tile([BL, H], F32, tag="oh")
    nc.vector.tensor_copy(out=oh_sb[:, :], in_=pso[0:BL, 0:H])
    nc.sync.dma_start(out=d["oh"][:, :], in_=oh_sb[:, :])


def _build():
    from contextlib import ExitStack
    nc = bacc.Bacc(None, target_bir_lowering=False)
    d = {
        "tok": nc.dram_tensor("tok", [R], I32, kind="ExternalInput"),
        "emb": nc.dram_tensor("emb", [V, H], F32, kind="ExternalInput"),
        "h0t": nc.dram_tensor("h0t", [H, BL], F32, kind="ExternalInput"),
        "wiht": nc.dram_tensor("wiht", [H, 3 * H], BF16, kind="ExternalInput"),
        "whht": nc.dram_tensor("whht", [H, 3 * H], BF16, kind="ExternalInput"),
        "bsc": nc.dram_tensor("bsc", [H, 4], F32, kind="ExternalInput"),
        "wot": nc.dram_tensor("wot", [H, V], BF16, kind="ExternalInput"),
        "bo": nc.dram_tensor("bo", [1, V], BF16, kind="ExternalInput"),
        "olp": nc.dram_tensor("olp", [R, V], F32, kind="ExternalOutput"),
        "oh": nc.dram_tensor("oh", [BL, H], F32, kind="ExternalOutput"),
    }
    with tile.TileContext(nc) as tc:
        with ExitStack() as ctx:
            _emit(nc, tc, ctx, d)
    nc.finalize()
    return nc


def _prepare_in_maps(encoder_hidden, target_tensor, emb,
                     W_ih, W_hh, b_ih, b_hh, W_out, b_out):
    bf16 = ml_dtypes.bfloat16
    tgt = np.asarray(target_tensor)
    tokens = np.concatenate(
        [np.zeros((B, 1), tgt.dtype), tgt[:, :-1]], axis=1).astype(np.int32)
    ehid = np.asarray(encoder_hidden, np.float32)
    embf = np.ascontiguousarray(np.asarray(emb, np.float32))
    b_ih = np.asarray(b_ih, np.float32)
    b_hh = np.asarray(b_hh, np.float32)
    wihT = np.ascontiguousarray(np.asarray(W_ih, np.float32).T).astype(bf16)
    whhT = np.ascontiguousarray(np.asarray(W_hh, np.float32).T).astype(bf16)
    woT = np.ascontiguousarray(np.asarray(W_out, np.float32).T).astype(bf16)
    bo = np.asarray(b_out, np.float32)[None, :].astype(bf16)
    bsc = np.ascontiguousarray(np.stack(
        [b_ih[0:H] + b_hh[0:H],
         b_ih[H:2 * H] + b_hh[H:2 * H],
         b_ih[2 * H:3 * H],
         0.5 * b_hh[2 * H:3 * H]], axis=1).astype(np.float32))

    in_maps = []
    for c in range(NCORES):
        bs = slice(BL * c, BL * (c + 1))
        tok_c = np.ascontiguousarray(tokens[bs].T).reshape(-1)  # t-major [R]
        h0T_c = np.ascontiguousarray(ehid[0, bs, :].T)          # [H, BL]
        in_maps.append({
            "tok": tok_c, "emb": embf, "h0t": h0T_c, "wiht": wihT,
            "whht": whhT, "bsc": bsc, "wot": woT, "bo": bo,
        })
    return in_maps


def _assemble(results):
    lp = np.empty((B, T, V), np.float32)
    hT = np.empty((1, B, H), np.float32)
    for c in range(NCORES):
        r = results[c]
        lp[BL * c:BL * (c + 1)] = \
            r["olp"].reshape(T, BL, V).transpose(1, 0, 2)
        hT[0, BL * c:BL * (c + 1)] = r["oh"]
    return lp, hT


def kernel(encoder_outputs, encoder_hidden, target_tensor, emb,
           W_ih, W_hh, b_ih, b_hh, W_out, b_out):
    global _NC
    if _NC is None:
        _NC = _build()
    in_maps = _prepare_in_maps(encoder_hidden, target_tensor, emb,
                               W_ih, W_hh, b_ih, b_hh, W_out, b_out)
    res = run_bass_kernel_spmd(_NC, in_maps, list(range(NCORES)))
    return _assemble(res.results)
